# revision 5
# baseline (speedup 1.0000x reference)
"""Chamfer distance loss on 8 Trainium2 NeuronCores.

Strategy (hardcoded for point clouds [1, 16384, 128] f32):
  - Shard point_cloud1 rows across 8 cores (2048 rows each); replicate
    point_cloud2.
  - Per core, PE computes psum tiles of dist-256 via two fp16 matmuls per
    512-chunk: one K=128 product pass (-2a.b, stationary = -2*a chunk) and
    one K=128-padded rank-2 pass ((a2-128)/ones | ones/(b2-128)).  fp16
    inputs keep the PE at 1 cycle/row and halve SBUF vs f32r.
  - ScalarE drains every psum group to a fp16 SBUF row TSB (Copy, 0.833
    ns/elem) -- the drain engine, nothing else.
  - VectorE (2x fp16 mode): direction-2 column mins accumulate into ACC
    (tensor_tensor min); direction-1 level-1 pair-min TSB -> S [128, 8192].
  - GpSimd: tree levels 8192->1024 on S (no PSUM port, SBUF only).
  - VectorE: tree tail 1024->256, then tensor_reduce -> RM[:, m].  The tree
    tail for chunk m is issued during chunk m+1 (software pipelining) so the
    in-order DVE queue never waits on the Pool chain.
  - Host: mean of row mins + mean over cores/partitions of column mins,
    +256 recenter.
"""
import numpy as np

N = 16384
D = 128
P = 128
NCORES = 8
MPC = N // NCORES          # rows per core = 2048
MCH = MPC // P             # row chunks per core = 16
NGRP = 8                   # column groups
GW = N // NGRP             # group width = 2048
HALF = N // 2              # 8192
CENTER = 256.0

_CACHE = {}


def _build(repeat=1):
    from contextlib import ExitStack, nullcontext
    import concourse.bacc as bacc
    import concourse.tile as tile
    from concourse import mybir

    f32 = mybir.dt.float32
    f16 = mybir.dt.float16
    MIN = mybir.AluOpType.min
    COPY = mybir.ActivationFunctionType.Copy

    nc = bacc.Bacc(trn_type="TRN2", target_bir_lowering=False, debug=False,
                   num_devices=NCORES)

    at_d = nc.dram_tensor("at", [D, MPC], f16, kind="ExternalInput").ap()
    bt_d = nc.dram_tensor("bt", [D, N], f16, kind="ExternalInput").ap()
    a2p_d = nc.dram_tensor("a2p", [D, MPC], f16, kind="ExternalInput").ap()
    ob2p_d = nc.dram_tensor("ob2p", [D, N], f16, kind="ExternalInput").ap()
    rm_d = nc.dram_tensor("rm", [P, MCH], f32, kind="ExternalOutput").ap()
    cm_d = nc.dram_tensor("cm", [P, N], f16, kind="ExternalOutput").ap()

    with tile.TileContext(nc) as tc, ExitStack() as ctx:
        cpool = ctx.enter_context(tc.tile_pool(name="const", bufs=1))
        psum_pool = ctx.enter_context(tc.tile_pool(name="psum", bufs=2, space="PSUM"))
        spool = ctx.enter_context(tc.tile_pool(name="s", bufs=2))

        AT = cpool.tile([D, MPC], f16)
        BT = cpool.tile([D, N], f16)
        A2P = cpool.tile([D, MPC], f16)
        OB2P = cpool.tile([D, N], f16)
        ACC = cpool.tile([P, N], f16)
        TSB = cpool.tile([P, N], f16)
        RM = cpool.tile([P, MCH], f32)

        nc.sync.dma_start(AT[:], at_d[:])
        nc.sync.dma_start(A2P[:], a2p_d[:])
        for g in range(NGRP):
            sl = slice(g * GW, (g + 1) * GW)
            nc.sync.dma_start(BT[:, sl], bt_d[:, sl])
            nc.sync.dma_start(OB2P[:, sl], ob2p_d[:, sl])

        def tree_tail(S, m):
            # 1024 -> 512 -> 256 on DVE, then reduce to RM[:, m]
            nc.vector.tensor_tensor(out=S[:, :512], in0=S[:, :512],
                                    in1=S[:, 512:1024], op=MIN)
            nc.vector.tensor_tensor(out=S[:, :256], in0=S[:, :256],
                                    in1=S[:, 256:512], op=MIN)
            nc.vector.tensor_reduce(out=RM[:, m:m + 1], in_=S[:, :256],
                                    axis=mybir.AxisListType.X, op=MIN)

        loop_ctx = tc.For_i(0, repeat, 1) if repeat > 1 else nullcontext()
        with loop_ctx:
            for m in range(MCH):
                msl = slice(m * P, (m + 1) * P)
                S = spool.tile([P, HALF], f16)  # [128, 8192]
                for g in range(NGRP):
                    ps = psum_pool.tile([P, GW], f32)
                    for k in range(4):
                        nsl = slice(g * GW + k * 512, g * GW + (k + 1) * 512)
                        ksl = slice(k * 512, (k + 1) * 512)
                        nc.tensor.matmul(ps[:, ksl], AT[:, msl], BT[:, nsl],
                                         start=True, stop=False)
                    for k in range(4):
                        nsl = slice(g * GW + k * 512, g * GW + (k + 1) * 512)
                        ksl = slice(k * 512, (k + 1) * 512)
                        nc.tensor.matmul(ps[:, ksl], A2P[:, msl], OB2P[:, nsl],
                                         start=False, stop=True)
                    gsl = slice(g * GW, (g + 1) * GW)
                    nc.scalar.activation(TSB[:, gsl], ps[:], COPY)
                    if g == 3 or g == 7:
                        h = slice(0, HALF) if g == 3 else slice(HALF, N)
                        if m == 0:
                            nc.vector.tensor_scalar_min(ACC[:, h], TSB[:, h],
                                                        60000.0)
                        else:
                            nc.vector.tensor_tensor(out=ACC[:, h],
                                                    in0=ACC[:, h],
                                                    in1=TSB[:, h], op=MIN)
                        # direction-1 level 1: pair columns j and j+4096
                        q = HALF // 2  # 4096
                        if g == 3:
                            nc.vector.tensor_tensor(out=S[:, :q],
                                                    in0=TSB[:, :q],
                                                    in1=TSB[:, q:HALF], op=MIN)
                        else:
                            nc.vector.tensor_tensor(out=S[:, q:],
                                                    in0=TSB[:, HALF:HALF + q],
                                                    in1=TSB[:, HALF + q:],
                                                    op=MIN)
                # tree levels 8192 -> 1024 on DVE (gpsimd lacks tensor min)
                nc.vector.tensor_tensor(out=S[:, :4096], in0=S[:, :4096],
                                        in1=S[:, 4096:], op=MIN)
                nc.vector.tensor_tensor(out=S[:, :2048], in0=S[:, :2048],
                                        in1=S[:, 2048:4096], op=MIN)
                nc.vector.tensor_tensor(out=S[:, :1024], in0=S[:, :1024],
                                        in1=S[:, 1024:2048], op=MIN)
                tree_tail(S, m)

        nc.sync.dma_start(rm_d[:], RM[:])
        for g in range(NGRP):
            sl = slice(g * GW, (g + 1) * GW)
            nc.sync.dma_start(cm_d[:, sl], ACC[:, sl])

    nc.compile()
    return nc


def _make_in_maps(pc1, pc2):
    a2 = (pc1.astype(np.float64) ** 2).sum(1).astype(np.float32)
    b2 = (pc2.astype(np.float64) ** 2).sum(1).astype(np.float32)

    bt = np.ascontiguousarray(pc2.T).astype(np.float16)
    ob2p = np.zeros((D, N), np.float16)
    ob2p[0] = 1.0
    ob2p[1] = (b2 - 128.0).astype(np.float16)

    in_maps = []
    for c in range(NCORES):
        rs = slice(c * MPC, (c + 1) * MPC)
        a2p = np.zeros((D, MPC), np.float16)
        a2p[0] = (a2[rs] - 128.0).astype(np.float16)
        a2p[1] = 1.0
        in_maps.append({
            "at": np.ascontiguousarray(-2.0 * pc1[rs].T).astype(np.float16),
            "bt": bt,
            "a2p": a2p,
            "ob2p": ob2p,
        })
    return in_maps


def kernel(point_cloud1: np.ndarray, point_cloud2: np.ndarray) -> np.ndarray:
    import os
    from concourse.bass_utils import run_bass_kernel_spmd

    if "nc" not in _CACHE:
        _CACHE["nc"] = _build()
    nc = _CACHE["nc"]

    pc1 = np.ascontiguousarray(np.asarray(point_cloud1).reshape(N, D),
                               dtype=np.float32)
    pc2 = np.ascontiguousarray(np.asarray(point_cloud2).reshape(N, D),
                               dtype=np.float32)
    in_maps = _make_in_maps(pc1, pc2)

    trace = os.environ.get("KERNEL_TRACE", "0") == "1"
    if trace:
        try:
            import antenv.axon_hooks  # noqa: F401
        except ImportError:
            trace = False
    res = run_bass_kernel_spmd(nc, in_maps, core_ids=list(range(NCORES)),
                               trace=trace)
    _CACHE["last_exec_ns"] = res.exec_time_ns

    rowmins = []
    colmins = []
    for r in res.results:
        # rm[p, m] is the row-min of core row m*128+p, minus CENTER
        rowmins.append(r["rm"].T.reshape(MPC))
        colmins.append(r["cm"].astype(np.float32))
    min1 = np.concatenate(rowmins) + CENTER
    min2 = np.concatenate(colmins, axis=0).min(axis=0) + CENTER
    out = np.float64(min1.mean()) + np.float64(min2.mean())
    return np.asarray(out, dtype=np.float32)
